# Initial kernel scaffold
#
"""EvolveGCN-H forward on 8 Trainium2 NeuronCores.

Strategy: destination-sharded aggregation. Nodes are sharded across the 8
cores; each core aggregates the messages of edges whose destination lands in
its shard via one-hot-mask matmuls accumulated in PSUM (edge payloads are
staged per-edge in HBM and streamed at line rate). The TopK scores are
computed sharded on-device, AllGathered, and the GRU weight evolution is
computed replicated on every core. Self-loops ride as extra edges.
"""
import numpy as np

N = 100000
E = 1600000
F = 128
H = 128
NC = 8
SHARD = N // NC            # 12500
SC_PAD = 12544             # 98*128, scores shard padding
V = NC * SC_PAD            # 100352 topk vocab
VHALF = V // 2             # 50176
CTILES = 25                # agg col tiles of 512
CPAD = CTILES * 512        # 12800
W_WIN = 32                 # mask col window


# ---------------------------------------------------------------- host prep
def _prep(inputs):
    x = np.asarray(inputs["x"], np.float32)
    ei = np.asarray(inputs["edge_index"], np.int64)
    Wm = np.asarray(inputs["W"], np.float32)
    p = np.asarray(inputs["p"], np.float32)
    w_ih = np.asarray(inputs["w_ih"], np.float32)
    w_hh = np.asarray(inputs["w_hh"], np.float32)
    b_ih = np.asarray(inputs["b_ih"], np.float32)
    b_hh = np.asarray(inputs["b_hh"], np.float32)
    conv_bias = np.asarray(inputs["conv_bias"], np.float32)
    lin_w = np.asarray(inputs["lin_w"], np.float32)
    lin_b = np.asarray(inputs["lin_b"], np.float32)

    x16 = x.astype(np.float16)
    row, col = ei[0], ei[1]
    deg = np.bincount(col, minlength=N).astype(np.int64) + 1

    # append self loops
    selfn = np.arange(N, dtype=np.int64)
    row_a = np.concatenate([row, selfn])
    col_a = np.concatenate([col, selfn])

    # scores-space node table (v-order): v = r*SC_PAD + p*98 + t,
    # node(r,p,t) = r*SHARD + t*128 + p  (locals >= SHARD are pads)
    vr = np.arange(V)
    rr, rem = vr // SC_PAD, vr % SC_PAD
    pp, tt = rem // 98, rem % 98
    loc = tt * 128 + pp
    node_of_v = rr * SHARD + np.minimum(loc, SHARD - 1)
    x_vtab = x[node_of_v] * (loc < SHARD)[:, None].astype(np.float32)

    per_core = []
    shard_meta = []
    for c in range(NC):
        m = (col_a >= c * SHARD) & (col_a < (c + 1) * SHARD)
        r_c = row_a[m]
        cl_c = col_a[m] - c * SHARD
        order = np.argsort(cl_c, kind="stable")
        r_c, cl_c = r_c[order], cl_c[order]

        # chunking: <=128 edges, col window <=W_WIN, within one 512 tile
        chunks = []          # (tile, c_lo_local, rows[128], cols_local[128], degr[128])
        i = 0
        M = len(r_c)
        while i < M:
            c0 = cl_c[i]
            tile = c0 // 512
            lo = min(c0 - tile * 512, 512 - W_WIN)
            lim_col = tile * 512 + lo + W_WIN
            j = min(i + 128, M)
            # shrink j until all cols < lim_col (cols sorted)
            j = i + np.searchsorted(cl_c[i:j], lim_col, side="left")
            nedge = j - i
            rows = np.zeros(128, np.int64)
            cols = np.zeros(128, np.int64)
            degr = np.zeros(128, np.int64)
            rows[:nedge] = r_c[i:j]
            cols[:nedge] = cl_c[i:j] - (tile * 512 + lo)
            degr[:nedge] = deg[r_c[i:j]]
            chunks.append((tile, lo, rows, cols, degr))
            i = j
        # group chunks by tile (already non-decreasing)
        tstart = [0] * (CTILES + 1)
        for (tile, *_rest) in chunks:
            tstart[tile + 1] += 1
        for t in range(CTILES):
            tstart[t + 1] += tstart[t]
        nch = len(chunks)
        rows_arr = np.stack([ch[2] for ch in chunks], 0)      # [nch,128]
        cols_arr = np.stack([ch[3] for ch in chunks], 0)
        degr_arr = np.stack([ch[4] for ch in chunks], 0)
        clos = [ch[1] for ch in chunks]
        tiles = [ch[0] for ch in chunks]

        pay = np.ascontiguousarray(
            x16[rows_arr].transpose(1, 0, 2))                  # [128,nch,128]
        colloc = np.ascontiguousarray(cols_arr.T.astype(np.float16))
        degrow = np.ascontiguousarray(degr_arr.T.astype(np.float16))

        degc = np.zeros(CPAD, np.float16)
        degc[:SHARD] = deg[c * SHARD:(c + 1) * SHARD].astype(np.float16)
        degc_rep = np.ascontiguousarray(np.broadcast_to(degc[None, :], (128, CPAD)))

        xT = np.zeros((128, SC_PAD), np.float32)
        xT[:, :SHARD] = x[c * SHARD:(c + 1) * SHARD].T

        per_core.append(dict(pay=pay, colloc=colloc, degrow=degrow,
                             degc=degc_rep, xT=xT))
        shard_meta.append(dict(nch=nch, clos=clos, tiles=tiles, tstart=tstart))

    consts = dict(
        x_vtab=x_vtab,
        w_sb=Wm,
        w_T=np.ascontiguousarray(Wm.T),
        w_ihT=np.ascontiguousarray(w_ih.T),
        w_hhT=np.ascontiguousarray(w_hh.T),
        b_ih_rep=np.ascontiguousarray(np.broadcast_to(b_ih[None, :], (128, 384))).astype(np.float32),
        b_hh_rep=np.ascontiguousarray(np.broadcast_to(b_hh[None, :], (128, 384))).astype(np.float32),
        cbias=conv_bias[:, None].astype(np.float32),
        lin_wT=lin_w[0][:, None].astype(np.float32),
        lin_b_rep=np.full((1, 4), lin_b[0], np.float32),
        ones_row=np.ones((1, 128), np.float32),
        iota3=np.ascontiguousarray(np.broadcast_to(
            np.arange(W_WIN, dtype=np.float16)[None, None, :], (128, 80, W_WIN))),
        iota128=np.ascontiguousarray(np.broadcast_to(
            np.arange(128, dtype=np.float32)[None, :], (128, 128))),
        ident=np.eye(128, dtype=np.float32),
        p_vec=p[:, None].astype(np.float32),
    )
    return per_core, shard_meta, consts


# ---------------------------------------------------------------- device prog
def _build(shard_meta, max_nch):
    import concourse.bass as bass
    import concourse.bacc as bacc
    import concourse.mybir as mybir
    import concourse.tile as tile
    import concourse.bass_isa as bass_isa
    from concourse.library_config import mlp

    dt32, dt16 = mybir.dt.float32, mybir.dt.float16
    AF = mybir.ActivationFunctionType
    OP = mybir.AluOpType
    nc = bacc.Bacc("TRN2", target_bir_lowering=False, debug=True)

    # per-core inputs
    pay_d = nc.dram_tensor("pay", [128, max_nch, F], dt16, kind="ExternalInput")
    coll_d = nc.dram_tensor("colloc", [128, max_nch], dt16, kind="ExternalInput")
    degr_d = nc.dram_tensor("degrow", [128, max_nch], dt16, kind="ExternalInput")
    degc_d = nc.dram_tensor("degc", [128, CPAD], dt16, kind="ExternalInput")
    xT_d = nc.dram_tensor("xT", [128, SC_PAD], dt32, kind="ExternalInput")
    # replicated consts
    xv_d = nc.dram_tensor("x_vtab", [V, F], dt32, kind="ExternalInput")
    wsb_d = nc.dram_tensor("w_sb", [128, 128], dt32, kind="ExternalInput")
    wT_d = nc.dram_tensor("w_T", [128, 128], dt32, kind="ExternalInput")
    wihT_d = nc.dram_tensor("w_ihT", [128, 384], dt32, kind="ExternalInput")
    whhT_d = nc.dram_tensor("w_hhT", [128, 384], dt32, kind="ExternalInput")
    bih_d = nc.dram_tensor("b_ih_rep", [128, 384], dt32, kind="ExternalInput")
    bhh_d = nc.dram_tensor("b_hh_rep", [128, 384], dt32, kind="ExternalInput")
    cb_d = nc.dram_tensor("cbias", [128, 1], dt32, kind="ExternalInput")
    lw_d = nc.dram_tensor("lin_wT", [128, 1], dt32, kind="ExternalInput")
    lb_d = nc.dram_tensor("lin_b_rep", [1, 4], dt32, kind="ExternalInput")
    ones_d = nc.dram_tensor("ones_row", [1, 128], dt32, kind="ExternalInput")
    iota3_d = nc.dram_tensor("iota3", [128, 80, W_WIN], dt16, kind="ExternalInput")
    io128_d = nc.dram_tensor("iota128", [128, 128], dt32, kind="ExternalInput")
    id_d = nc.dram_tensor("ident", [128, 128], dt32, kind="ExternalInput")
    p_d = nc.dram_tensor("p_vec", [128, 1], dt32, kind="ExternalInput")

    out_d = nc.dram_tensor("out", [128, CTILES * 4], dt32, kind="ExternalOutput")

    meta0 = shard_meta  # per-core metadata must be IDENTICAL program; use core0's
    with tile.TileContext(nc) as tc:
        with (
            tc.tile_pool(name="cst", bufs=1) as cst,
            tc.tile_pool(name="sc", bufs=1) as scp,
            tc.tile_pool(name="pay", bufs=3) as payp,
            tc.tile_pool(name="msk", bufs=3) as mskp,
            tc.tile_pool(name="work", bufs=1) as wk,
            tc.tile_pool(name="ps", bufs=2, space="PSUM") as ps,
            tc.tile_pool(name="ps1", bufs=1, space="PSUM") as ps1,
            tc.tile_pool(name="dram", bufs=1, space="DRAM") as dram,
        ):
            nc.gpsimd.load_library(mlp)

            # ---- load consts
            wsb = cst.tile([128, 128], dt32)
            wT = cst.tile([128, 128], dt32)
            wihT = cst.tile([128, 384], dt32)
            whhT = cst.tile([128, 384], dt32)
            bih = cst.tile([128, 384], dt32)
            bhh = cst.tile([128, 384], dt32)
            cb = cst.tile([128, 1], dt32)
            lw = cst.tile([128, 1], dt32)
            lb = cst.tile([1, 4], dt32)
            ones = cst.tile([1, 128], dt32)
            iota3 = cst.tile([128, 80, W_WIN], dt16)
            io128 = cst.tile([128, 128], dt32)
            ident = cst.tile([128, 128], dt32)
            pv = cst.tile([128, 1], dt32)
            for t, d in [(wsb, wsb_d), (wT, wT_d), (wihT, wihT_d), (whhT, whhT_d),
                         (bih, bih_d), (bhh, bhh_d), (cb, cb_d), (lw, lw_d),
                         (lb, lb_d), (ones, ones_d), (iota3, iota3_d),
                         (io128, io128_d), (ident, id_d), (pv, p_d)]:
                nc.sync.dma_start(t[:], d[:])

            # ---- scores: ||p|| then x @ p/||p|| for own shard
            psn = ps1.tile([1, 1], dt32, tag="psn")
            nc.tensor.matmul(out=psn[:], lhsT=pv[:], rhs=pv[:], start=True, stop=True)
            sqn = wk.tile([1, 1], dt32)
            nc.scalar.activation(sqn[:], psn[:], AF.Sqrt)
            rn = wk.tile([1, 1], dt32)
            nc.vector.reciprocal(rn[:], sqn[:])
            ps_rep = ps1.tile([128, 1], dt32, tag="psrep")
            nc.tensor.matmul(out=ps_rep[:], lhsT=ones[:], rhs=rn[:], start=True, stop=True)
            pscl = wk.tile([128, 1], dt32)
            nc.vector.tensor_tensor(out=pscl[:], in0=ps_rep[:], in1=pv[:], op=OP.mult)

            xT = scp.tile([128, SC_PAD], dt32)
            nc.sync.dma_start(xT[:], xT_d[:])
            scsb = wk.tile([128, 98], dt32)
            for g in range(25):
                nsub = min(4, 98 - g * 4)
                pssc = ps.tile([128, 4], dt32, tag="pssc")
                for msub in range(nsub):
                    t = g * 4 + msub
                    nc.tensor.matmul(out=pssc[:, msub:msub + 1],
                                     lhsT=xT[:, t * 128:(t + 1) * 128],
                                     rhs=pscl[:], start=True, stop=True,
                                     skip_group_check=True)
                nc.vector.tensor_copy(scsb[:, g * 4:g * 4 + nsub], pssc[:, :nsub])

            sc_dram = dram.tile([128, 98], dt32)
            nc.sync.dma_start(sc_dram[:], scsb[:])
            ag_out = dram.tile([V], dt32)
            nc.gpsimd.collective_compute(
                "AllGather", mybir.AluOpType.bypass,
                replica_groups=[list(range(NC))],
                ins=[sc_dram.opt()], outs=[ag_out.opt()])

            # ---- topk(2 x 50176) -> merge 512 candidates by rank
            tkin = wk.tile([32, V // 32], dt32)
            nc.sync.dma_start(tkin[:], ag_out[:].rearrange("(p a) -> p a", p=32))
            tk = wk.tile([32, 32], mybir.dt.uint32)
            g = nc.gpsimd
            g.add_instruction(bass_isa.InstTopk(
                name=f"I-{nc.next_id()}",
                ins=[g.lower_ap(tkin[:], for_isa=True)],
                outs=[g.lower_ap(tk[:], for_isa=True)],
                _tokens=2, _n=VHALF, _k=256))

            # unfold vals [16,16]x2 -> cv [128,4]; idx -> ci [128,4] int32
            cv = wk.tile([128, 4], dt32)
            ci = wk.tile([128, 4], mybir.dt.int32)
            cvrow = wk.tile([1, 512], dt32)
            for half in range(2):
                for gcol in range(2):
                    src_v = tk[half * 16 + gcol * 8: half * 16 + (gcol + 1) * 8, 0:16]
                    src_i = tk[half * 16 + gcol * 8: half * 16 + (gcol + 1) * 8, 16:32]
                    cidx = half * 2 + gcol
                    nc.sync.dma_start(cv[:, cidx:cidx + 1], src_v.bitcast(dt32))
                    nc.sync.dma_start(ci[:, cidx:cidx + 1], src_i.bitcast(mybir.dt.int32))
                nc.sync.dma_start(cvrow[:, half * 256:(half + 1) * 256],
                                  tk[half * 16:(half + 1) * 16, 0:16].bitcast(dt32))
            nc.vector.tensor_scalar_add(ci[:, 2:4], ci[:, 2:4], VHALF)

            ps_vr = ps1.tile([128, 512], dt32, tag="psvr")
            nc.tensor.matmul(out=ps_vr[:], lhsT=ones[:], rhs=cvrow[:], start=True, stop=True)
            vrep = wk.tile([128, 512], dt32)
            nc.vector.tensor_copy(vrep[:], ps_vr[:])

            ranks = wk.tile([128, 4], dt32)
            gtt = wk.tile([128, 512], dt32)
            for gc in range(4):
                nc.vector.tensor_scalar(out=gtt[:], in0=vrep[:],
                                        scalar1=cv[:, gc:gc + 1], scalar2=None,
                                        op0=OP.is_gt)
                nc.vector.reduce_sum(ranks[:, gc:gc + 1], gtt[:],
                                     axis=mybir.AxisListType.X)

            ps_xt = ps1.tile([128, 128], dt32, tag="psxt")
            ps_vb = ps1.tile([128, 1], dt32, tag="psvb")
            for gc in range(4):
                oh = wk.tile([128, 128], dt32, tag="oh")
                nc.vector.tensor_scalar(out=oh[:], in0=io128[:],
                                        scalar1=ranks[:, gc:gc + 1], scalar2=None,
                                        op0=OP.is_equal)
                xc = wk.tile([128, 128], dt32, tag="xc")
                nc.gpsimd.indirect_dma_start(
                    out=xc[:], out_offset=None, in_=xv_d[:],
                    in_offset=bass.IndirectOffsetOnAxis(ap=ci[:, gc:gc + 1], axis=0))
                nc.tensor.matmul(out=ps_xt[:], lhsT=oh[:], rhs=xc[:],
                                 start=(gc == 0), stop=(gc == 3), skip_group_check=True)
                nc.tensor.matmul(out=ps_vb[:], lhsT=oh[:], rhs=cv[:, gc:gc + 1],
                                 start=(gc == 0), stop=(gc == 3), skip_group_check=True)
            tanhv = wk.tile([128, 1], dt32)
            nc.scalar.activation(tanhv[:], ps_vb[:], AF.Tanh)
            xtl = wk.tile([128, 128], dt32)
            nc.vector.tensor_scalar_mul(xtl[:], ps_xt[:], tanhv[:, :1])

            # ---- GRU step
            ps_t = ps1.tile([128, 128], dt32, tag="pst")
            nc.tensor.transpose(out=ps_t[:], in_=xtl[:], identity=ident[:])
            xtT = wk.tile([128, 128], dt32)
            nc.vector.tensor_copy(xtT[:], ps_t[:])
            ps_gi = ps1.tile([128, 384], dt32, tag="psgi")
            nc.tensor.matmul(out=ps_gi[:], lhsT=xtT[:], rhs=wihT[:], start=True, stop=True)
            ps_gh = ps1.tile([128, 384], dt32, tag="psgh")
            nc.tensor.matmul(out=ps_gh[:], lhsT=wT[:], rhs=whhT[:], start=True, stop=True)
            gi = wk.tile([128, 384], dt32)
            nc.vector.tensor_add(gi[:], ps_gi[:], bih[:])
            gh = wk.tile([128, 384], dt32)
            nc.vector.tensor_add(gh[:], ps_gh[:], bhh[:])
            rz = wk.tile([128, 256], dt32)
            nc.vector.tensor_add(rz[:], gi[:, 0:256], gh[:, 0:256])
            nc.scalar.activation(rz[:], rz[:], AF.Sigmoid)
            nn_ = wk.tile([128, 128], dt32)
            nc.vector.tensor_mul(nn_[:], rz[:, 0:128], gh[:, 256:384])
            nc.vector.tensor_add(nn_[:], nn_[:], gi[:, 256:384])
            nc.scalar.activation(nn_[:], nn_[:], AF.Tanh)
            # W_new = (1-z)*n + z*W = n + z*(W - n)
            wmn = wk.tile([128, 128], dt32)
            nc.vector.tensor_sub(wmn[:], wsb[:], nn_[:])
            nc.vector.tensor_mul(wmn[:], rz[:, 128:256], wmn[:])
            wnew = wk.tile([128, 128], dt32)
            nc.vector.tensor_add(wnew[:], nn_[:], wmn[:])

            # ---- dinv tables
            with nc.allow_low_precision(reason="dinv fp16 is plenty"):
                dcsb = scp.tile([128, CPAD], dt16)
                nc.sync.dma_start(dcsb[:], degc_d[:])
                nc.scalar.activation(dcsb[:], dcsb[:], AF.Sqrt, bias=1.0)
                nc.vector.reciprocal(dcsb[:], dcsb[:])
                colloc = scp.tile([128, max_nch], dt16)
                nc.sync.dma_start(colloc[:], coll_d[:])
                dgr = scp.tile([128, max_nch], dt16)
                nc.sync.dma_start(dgr[:], degr_d[:])
                nc.scalar.activation(dgr[:], dgr[:], AF.Sqrt, bias=1.0)
                nc.vector.reciprocal(dgr[:], dgr[:])

            # ---- aggregation over col tiles
            outsb = wk.tile([128, CTILES * 4], dt32)
            tstart, clos = meta0["tstart"], meta0["clos"]
            for T in range(CTILES):
                ch0, ch1 = tstart[T], tstart[T + 1]
                nch = ch1 - ch0
                pspre = ps.tile([128, 512], dt32, tag="pspre")
                nc.vector.memset(pspre[:], 0.0)
                if nch > 0:
                    payt = payp.tile([128, max(nch, 1), F], dt16, tag="payt")
                    nc.sync.dma_start(payt[:, :nch, :], pay_d[:, ch0:ch1, :])
                    mv = mskp.tile([128, max(nch, 1), W_WIN], dt16, tag="mv")
                    nc.vector.tensor_tensor(
                        out=mv[:, :nch, :],
                        in0=colloc[:, ch0:ch1].to_broadcast([128, nch, W_WIN]),
                        in1=iota3[:, :nch, :], op=OP.is_equal)
                    nc.vector.tensor_tensor(
                        out=mv[:, :nch, :], in0=mv[:, :nch, :],
                        in1=dgr[:, ch0:ch1].to_broadcast([128, nch, W_WIN]),
                        op=OP.mult)
                    for k in range(nch):
                        lo = clos[ch0 + k]
                        nc.tensor.matmul(out=pspre[:, lo:lo + W_WIN],
                                         lhsT=payt[:, k, :], rhs=mv[:, k, :],
                                         start=False, stop=(k == nch - 1),
                                         skip_group_check=True)
                aggpre = wk.tile([128, 512], dt32, tag="aggpre")
                nc.vector.tensor_tensor(out=aggpre[:], in0=pspre[:],
                                        in1=dcsb[:, T * 512:(T + 1) * 512],
                                        op=OP.mult)
                psagg = ps.tile([128, 512], dt32, tag="psagg")
                nc.tensor.matmul(out=psagg[:], lhsT=wnew[:], rhs=aggpre[:],
                                 start=True, stop=True)
                relu = wk.tile([128, 512], dt32, tag="relu")
                nc.scalar.activation(relu[:], psagg[:], AF.Relu, bias=cb[:])
                pshead = ps.tile([128, 4], dt32, tag="pshead")
                nc.tensor.matmul(out=pshead[:], lhsT=ones[:], rhs=lb[:],
                                 start=True, stop=False, skip_group_check=True)
                for m in range(4):
                    nc.tensor.matmul(out=pshead[:, m:m + 1],
                                     lhsT=relu[:, m * 128:(m + 1) * 128],
                                     rhs=lw[:], start=False, stop=(m == 3),
                                     skip_group_check=True)
                nc.vector.tensor_copy(outsb[:, T * 4:(T + 1) * 4], pshead[:])

            nc.sync.dma_start(out_d[:], outsb[:])

    nc.compile()
    return nc


# ---------------------------------------------------------------- runner
def _run(nc, in_maps):
    import jax
    import numpy as _np
    from jax.sharding import Mesh, PartitionSpec, NamedSharding
    from jax.experimental.shard_map import shard_map
    import concourse.mybir as mybir
    from concourse import bass2jax
    from concourse.bass2jax import _bass_exec_p, install_neuronx_cc_hook

    install_neuronx_cc_hook()
    partition_name = nc.partition_id_tensor.name if nc.partition_id_tensor else None
    in_names, out_names, out_avals, zero_outs = [], [], [], []
    for alloc in nc.m.functions[0].allocations:
        if not isinstance(alloc, mybir.MemoryLocationSet):
            continue
        name = alloc.memorylocations[0].name
        if alloc.kind == "ExternalInput":
            if name != partition_name:
                in_names.append(name)
        elif alloc.kind == "ExternalOutput":
            out_names.append(name)
            shape = tuple(alloc.tensor_shape)
            dtype = mybir.dt.np(alloc.dtype)
            out_avals.append(jax.core.ShapedArray(shape, dtype))
            zero_outs.append(_np.zeros(shape, dtype))
    n_params = len(in_names)
    n_outs = len(out_avals)
    all_in = list(in_names) + list(out_names)
    if partition_name is not None:
        all_in.append(partition_name)
    donate = tuple(range(n_params, n_params + n_outs))

    def _body(*args):
        operands = list(args)
        if partition_name is not None:
            operands.append(bass2jax.partition_id_tensor())
        return tuple(_bass_exec_p.bind(
            *operands, out_avals=tuple(out_avals), in_names=tuple(all_in),
            out_names=tuple(out_names), lowering_input_output_aliases=(),
            sim_require_finite=False, sim_require_nnan=False, nc=nc))

    devices = jax.devices()[:NC]
    mesh = Mesh(_np.asarray(devices), ("core",))
    sharded = jax.jit(
        shard_map(_body, mesh=mesh,
                  in_specs=(PartitionSpec("core"),) * (n_params + n_outs),
                  out_specs=(PartitionSpec("core"),) * len(out_names),
                  check_rep=False),
        donate_argnums=donate, keep_unused=True)

    dbg = nc.dbg_addr.name if nc.dbg_addr is not None else None
    if dbg is not None:
        in_maps = [{**m, dbg: _np.zeros((1, 2), _np.uint32)} for m in in_maps]
    per_core = [[_np.asarray(m[nm]) for nm in in_names] for m in in_maps]
    concat_in = [
        _np.ascontiguousarray(_np.concatenate(
            [per_core[c][i] for c in range(NC)], axis=0))
        for i in range(n_params)]
    concat_zeros = [_np.zeros((NC * z.shape[0], *z.shape[1:]), z.dtype)
                    for z in zero_outs]
    out_arrs = sharded(*concat_in, *concat_zeros)
    return [
        {name: _np.asarray(out_arrs[i]).reshape(NC, *out_avals[i].shape)[c]
         for i, name in enumerate(out_names)}
        for c in range(NC)]


_CACHE = {}


def kernel(**inputs) -> np.ndarray:
    per_core, shard_meta, consts = _prep(inputs)
    max_nch = max(m["nch"] for m in shard_meta)

    # identical program across cores requires identical chunk structure ->
    # pad every core's metadata to core0's? Program uses core0 meta; other
    # cores MUST have same nch/tstart/clos. Enforce via host padding:
    # (we rebuild per-core arrays to the common chunk plan below)
    nc = _build(shard_meta[0], max_nch)

    in_maps = []
    for c in range(NC):
        m = dict(per_core[c])
        # pad pay/colloc/degrow to max_nch
        for k, fill in (("pay", 0), ("colloc", 0), ("degrow", 0)):
            a = m[k]
            if a.shape[1] < max_nch:
                pad = [(0, 0)] * a.ndim
                pad[1] = (0, max_nch - a.shape[1])
                m[k] = np.pad(a, pad)
        m.update(consts)
        in_maps.append(m)

    res = _run(nc, in_maps)
    out = np.empty((N, 1), np.float32)
    for c in range(NC):
        o = res[c]["out"]                      # [128, 100]
        out[c * SHARD:(c + 1) * SHARD, 0] = o.T.reshape(-1)[:SHARD]
    return out


# revision 8
# speedup vs baseline: 3.0642x; 3.0642x over previous
"""EvolveGCN-H forward on 8 Trainium2 NeuronCores.

Strategy: destination-sharded aggregation. Nodes are sharded across the 8
cores; each core aggregates the messages of edges whose destination lands in
its shard via one-hot-mask matmuls accumulated in PSUM (edge payloads are
staged per-edge in HBM and streamed at line rate). The TopK scores are
computed sharded on-device, AllGathered, and the GRU weight evolution is
computed replicated on every core. Self-loops ride as extra edges.
"""
import numpy as np

N = 100000
E = 1600000
F = 128
H = 128
NC = 8
SHARD = N // NC            # 12500
SC_PAD = 12544             # 98*128, scores shard padding
V = NC * SC_PAD            # 100352 topk vocab
VHALF = V // 2             # 50176
CTILES = 25                # agg col tiles of 512
CPAD = CTILES * 512        # 12800
W_WIN = 32                 # mask col window


# ---------------------------------------------------------------- host prep
def _prep(inputs):
    x = np.asarray(inputs["x"], np.float32)
    ei = np.asarray(inputs["edge_index"], np.int64)
    Wm = np.asarray(inputs["W"], np.float32)
    p = np.asarray(inputs["p"], np.float32)
    w_ih = np.asarray(inputs["w_ih"], np.float32)
    w_hh = np.asarray(inputs["w_hh"], np.float32)
    b_ih = np.asarray(inputs["b_ih"], np.float32)
    b_hh = np.asarray(inputs["b_hh"], np.float32)
    conv_bias = np.asarray(inputs["conv_bias"], np.float32)
    lin_w = np.asarray(inputs["lin_w"], np.float32)
    lin_b = np.asarray(inputs["lin_b"], np.float32)

    x16 = x.astype(np.float16)
    row, col = ei[0], ei[1]
    # staged as raw in-degree counts; device computes 1/sqrt(count + 1)
    deg = np.bincount(col, minlength=N).astype(np.int64)

    # append self loops
    selfn = np.arange(N, dtype=np.int64)
    row_a = np.concatenate([row, selfn])
    col_a = np.concatenate([col, selfn])

    # scores-space node table (v-order): v = r*SC_PAD + p*98 + t,
    # node(r,p,t) = r*SHARD + t*128 + p  (locals >= SHARD are pads)
    vr = np.arange(V)
    rr, rem = vr // SC_PAD, vr % SC_PAD
    pp, tt = rem // 98, rem % 98
    loc = tt * 128 + pp
    node_of_v = rr * SHARD + np.minimum(loc, SHARD - 1)
    x_vtab = x[node_of_v] * (loc < SHARD)[:, None].astype(np.float32)

    # fixed 32-col windows: cell = local_col // 32 (400 cells per core),
    # K chunks of 128 edge-slots per cell, K identical across cores.
    NCELL = CPAD // W_WIN  # 400
    core_edges = []
    K = 1
    for c in range(NC):
        m = (col_a >= c * SHARD) & (col_a < (c + 1) * SHARD)
        r_c = row_a[m]
        cl_c = col_a[m] - c * SHARD
        cell = cl_c // W_WIN
        order = np.argsort(cell, kind="stable")
        r_c, cl_c, cell = r_c[order], cl_c[order], cell[order]
        cnt = np.bincount(cell, minlength=NCELL)
        K = max(K, int(np.ceil(cnt.max() / 128)))
        core_edges.append((r_c, cl_c, cell, cnt))

    per_core = []
    for c in range(NC):
        r_c, cl_c, cell, cnt = core_edges[c]
        starts = np.zeros(NCELL, np.int64)
        starts[1:] = np.cumsum(cnt)[:-1]
        off = np.arange(len(r_c)) - starts[cell]
        slot = (cell * K + off // 128) * 128 + off % 128
        TOT = NCELL * K * 128
        rows_f = np.zeros(TOT, np.int64)
        cols_f = np.zeros(TOT, np.int64)
        degr_f = np.zeros(TOT, np.int64)
        valid = np.zeros(TOT, bool)
        rows_f[slot] = r_c
        cols_f[slot] = cl_c % W_WIN
        degr_f[slot] = deg[r_c]
        valid[slot] = True

        pay = x16[rows_f] * valid[:, None]
        pay = np.ascontiguousarray(
            pay.reshape(NCELL * K, 128, F).transpose(1, 0, 2))   # [128,nch,128]
        colloc = np.ascontiguousarray(
            cols_f.reshape(NCELL * K, 128).T.astype(np.float16))
        degrow = np.ascontiguousarray(
            (degr_f * valid).reshape(NCELL * K, 128).T.astype(np.float16))

        degc = np.zeros(CPAD, np.float16)
        degc[:SHARD] = deg[c * SHARD:(c + 1) * SHARD].astype(np.float16)
        degc_rep = np.ascontiguousarray(np.broadcast_to(degc[None, :], (128, CPAD)))

        xT = np.zeros((128, SC_PAD), np.float32)
        xT[:, :SHARD] = x[c * SHARD:(c + 1) * SHARD].T

        per_core.append(dict(pay=pay, colloc=colloc, degrow=degrow,
                             degc=degc_rep, xT=xT))

    consts = dict(
        K=K,
        x_vtab=x_vtab,
        w_sb=Wm,
        w_T=np.ascontiguousarray(Wm.T),
        w_ihT=np.ascontiguousarray(w_ih.T),
        w_hhT=np.ascontiguousarray(w_hh.T),
        b_ih_rep=np.ascontiguousarray(np.broadcast_to(b_ih[None, :], (128, 384))).astype(np.float32),
        b_hh_rep=np.ascontiguousarray(np.broadcast_to(b_hh[None, :], (128, 384))).astype(np.float32),
        cbias=conv_bias[:, None].astype(np.float32),
        lin_wT=lin_w[0][:, None].astype(np.float32),
        lin_b_rep=np.full((1, 4), lin_b[0], np.float32),
        ones_row=np.ones((1, 128), np.float32),
        iota3=np.ascontiguousarray(np.broadcast_to(
            np.arange(W_WIN, dtype=np.float16)[None, None, :],
            (128, 16 * K, W_WIN))),
        iota128=np.ascontiguousarray(np.broadcast_to(
            np.arange(128, dtype=np.float32)[None, :], (128, 128))),
        ident=np.eye(128, dtype=np.float32),
        p_vec=p[:, None].astype(np.float32),
    )
    return per_core, consts


# ---------------------------------------------------------------- device prog
def _build(K, empty=False, agg_reps=1):
    max_nch = (CPAD // W_WIN) * K
    tile_nch = 16 * K
    import concourse.bass as bass
    import concourse.bacc as bacc
    import concourse.mybir as mybir
    import concourse.tile as tile
    import concourse.bass_isa as bass_isa
    from concourse.library_config import mlp

    dt32, dt16 = mybir.dt.float32, mybir.dt.float16
    AF = mybir.ActivationFunctionType
    OP = mybir.AluOpType
    nc = bacc.Bacc("TRN2", target_bir_lowering=False, debug=True)

    # per-core inputs
    pay_d = nc.dram_tensor("pay", [128, max_nch, F], dt16, kind="ExternalInput")
    coll_d = nc.dram_tensor("colloc", [128, max_nch], dt16, kind="ExternalInput")
    degr_d = nc.dram_tensor("degrow", [128, max_nch], dt16, kind="ExternalInput")
    degc_d = nc.dram_tensor("degc", [128, CPAD], dt16, kind="ExternalInput")
    xT_d = nc.dram_tensor("xT", [128, SC_PAD], dt32, kind="ExternalInput")
    # replicated consts
    xv_d = nc.dram_tensor("x_vtab", [V, F], dt32, kind="ExternalInput")
    wsb_d = nc.dram_tensor("w_sb", [128, 128], dt32, kind="ExternalInput")
    wT_d = nc.dram_tensor("w_T", [128, 128], dt32, kind="ExternalInput")
    wihT_d = nc.dram_tensor("w_ihT", [128, 384], dt32, kind="ExternalInput")
    whhT_d = nc.dram_tensor("w_hhT", [128, 384], dt32, kind="ExternalInput")
    bih_d = nc.dram_tensor("b_ih_rep", [128, 384], dt32, kind="ExternalInput")
    bhh_d = nc.dram_tensor("b_hh_rep", [128, 384], dt32, kind="ExternalInput")
    cb_d = nc.dram_tensor("cbias", [128, 1], dt32, kind="ExternalInput")
    lw_d = nc.dram_tensor("lin_wT", [128, 1], dt32, kind="ExternalInput")
    lb_d = nc.dram_tensor("lin_b_rep", [1, 4], dt32, kind="ExternalInput")
    ones_d = nc.dram_tensor("ones_row", [1, 128], dt32, kind="ExternalInput")
    iota3_d = nc.dram_tensor("iota3", [128, tile_nch, W_WIN], dt16, kind="ExternalInput")
    io128_d = nc.dram_tensor("iota128", [128, 128], dt32, kind="ExternalInput")
    id_d = nc.dram_tensor("ident", [128, 128], dt32, kind="ExternalInput")
    p_d = nc.dram_tensor("p_vec", [128, 1], dt32, kind="ExternalInput")

    out_d = nc.dram_tensor("out", [128, CTILES * 4], dt32, kind="ExternalOutput")

    if empty:
        with tile.TileContext(nc) as tc:
            with tc.tile_pool(name="wk", bufs=1) as wk:
                z = wk.tile([128, CTILES * 4], dt32)
                nc.sync.dma_start(z[:], xT_d[:, :CTILES * 4])
                nc.sync.dma_start(out_d[:], z[:])
        nc.compile()
        return nc
    with tile.TileContext(nc) as tc:
        with (
            tc.tile_pool(name="cst", bufs=1) as cst,
            tc.tile_pool(name="sc", bufs=1) as scp,
            tc.tile_pool(name="pay", bufs=3) as payp,
            tc.tile_pool(name="msk", bufs=3) as mskp,
            tc.tile_pool(name="work", bufs=1) as wk,
            tc.tile_pool(name="ps", bufs=2, space="PSUM") as ps,
            tc.tile_pool(name="ps1", bufs=1, space="PSUM") as ps1,
            tc.tile_pool(name="dram", bufs=1, space="DRAM") as dram,
        ):
            nc.gpsimd.load_library(mlp)

            # ---- load consts
            wsb = cst.tile([128, 128], dt32)
            wT = cst.tile([128, 128], dt32)
            wihT = cst.tile([128, 384], dt32)
            whhT = cst.tile([128, 384], dt32)
            bih = cst.tile([128, 384], dt32)
            bhh = cst.tile([128, 384], dt32)
            cb = cst.tile([128, 1], dt32)
            lw = cst.tile([128, 1], dt32)
            lb = cst.tile([1, 4], dt32)
            ones = cst.tile([1, 128], dt32)
            iota3 = cst.tile([128, tile_nch, W_WIN], dt16)
            io128 = cst.tile([128, 128], dt32)
            ident = cst.tile([128, 128], dt32)
            pv = cst.tile([128, 1], dt32)
            for t, d in [(wsb, wsb_d), (wT, wT_d), (wihT, wihT_d), (whhT, whhT_d),
                         (bih, bih_d), (bhh, bhh_d), (cb, cb_d), (lw, lw_d),
                         (lb, lb_d), (ones, ones_d), (iota3, iota3_d),
                         (io128, io128_d), (ident, id_d), (pv, p_d)]:
                nc.sync.dma_start(t[:], d[:])

            # ---- scores: ||p|| then x @ p/||p|| for own shard
            psn = ps1.tile([1, 1], dt32, tag="p1")
            nc.tensor.matmul(out=psn[:], lhsT=pv[:], rhs=pv[:], start=True, stop=True)
            sqn = wk.tile([1, 1], dt32)
            nc.scalar.activation(sqn[:], psn[:], AF.Sqrt)
            rn = wk.tile([1, 1], dt32)
            nc.vector.reciprocal(rn[:], sqn[:])
            ps_rep = ps1.tile([128, 1], dt32, tag="p2")
            nc.tensor.matmul(out=ps_rep[:], lhsT=ones[:], rhs=rn[:], start=True, stop=True)
            pscl = wk.tile([128, 1], dt32)
            nc.vector.tensor_tensor(out=pscl[:], in0=ps_rep[:], in1=pv[:], op=OP.mult)

            xT = scp.tile([128, SC_PAD], dt32)
            nc.sync.dma_start(xT[:], xT_d[:])
            scsb = wk.tile([128, 98], dt32)
            for g in range(25):
                nsub = min(4, 98 - g * 4)
                pssc = ps.tile([128, 4], dt32, tag="ps4")
                for msub in range(nsub):
                    t = g * 4 + msub
                    nc.tensor.matmul(out=pssc[:, msub:msub + 1],
                                     lhsT=xT[:, t * 128:(t + 1) * 128],
                                     rhs=pscl[:], start=True, stop=True,
                                     skip_group_check=True)
                nc.vector.tensor_copy(scsb[:, g * 4:g * 4 + nsub], pssc[:, :nsub])

            sc_dram = dram.tile([128, 98], dt32)
            nc.sync.dma_start(sc_dram[:], scsb[:])
            ag_out = dram.tile([V], dt32)
            nc.gpsimd.collective_compute(
                "AllGather", mybir.AluOpType.bypass,
                replica_groups=[list(range(NC))],
                ins=[sc_dram.opt()], outs=[ag_out.opt()])

            # ---- topk(2 x 50176) -> merge 512 candidates by rank
            tkin = wk.tile([32, V // 32], dt32)
            nc.sync.dma_start(tkin[:], ag_out[:].rearrange("(p a) -> p a", p=32))
            tk = wk.tile([32, 32], mybir.dt.uint32)
            g = nc.gpsimd
            g.add_instruction(bass_isa.InstTopk(
                name=f"I-{nc.next_id()}",
                ins=[g.lower_ap(tkin[:], for_isa=True)],
                outs=[g.lower_ap(tk[:], for_isa=True)],
                _tokens=2, _n=VHALF, _k=256))

            # unfold vals [16,16]x2 -> cv [128,4]; idx -> ci [128,4] int32
            cv = wk.tile([128, 4], dt32)
            ci = wk.tile([128, 4], mybir.dt.int32)
            cvrow = wk.tile([1, 512], dt32)
            for half in range(2):
                for gcol in range(2):
                    src_v = tk[half * 16 + gcol * 8: half * 16 + (gcol + 1) * 8, 0:16]
                    src_i = tk[half * 16 + gcol * 8: half * 16 + (gcol + 1) * 8, 16:32]
                    cidx = half * 2 + gcol
                    nc.sync.dma_start(cv[:, cidx:cidx + 1], src_v.bitcast(dt32))
                    nc.sync.dma_start(ci[:, cidx:cidx + 1], src_i.bitcast(mybir.dt.int32))
                nc.sync.dma_start(cvrow[:, half * 256:(half + 1) * 256],
                                  tk[half * 16:(half + 1) * 16, 0:16].bitcast(dt32))
            nc.vector.tensor_scalar_add(ci[:, 2:4], ci[:, 2:4], VHALF)

            ps_vr = ps1.tile([128, 512], dt32, tag="p1")
            nc.tensor.matmul(out=ps_vr[:], lhsT=ones[:], rhs=cvrow[:], start=True, stop=True)
            vrep = wk.tile([128, 512], dt32)
            nc.vector.tensor_copy(vrep[:], ps_vr[:])

            ranks = wk.tile([128, 4], dt32)
            gtt = wk.tile([128, 512], dt32)
            for gc in range(4):
                nc.vector.tensor_scalar(out=gtt[:], in0=vrep[:],
                                        scalar1=cv[:, gc:gc + 1], scalar2=None,
                                        op0=OP.is_gt)
                nc.vector.reduce_sum(ranks[:, gc:gc + 1], gtt[:],
                                     axis=mybir.AxisListType.X)

            ps_xt = ps1.tile([128, 128], dt32, tag="p1")
            ps_vb = ps1.tile([128, 1], dt32, tag="p2")
            for gc in range(4):
                oh = wk.tile([128, 128], dt32, tag="oh")
                nc.vector.tensor_scalar(out=oh[:], in0=io128[:],
                                        scalar1=ranks[:, gc:gc + 1], scalar2=None,
                                        op0=OP.is_equal)
                xc = wk.tile([128, 128], dt32, tag="xc")
                nc.gpsimd.indirect_dma_start(
                    out=xc[:], out_offset=None, in_=xv_d[:],
                    in_offset=bass.IndirectOffsetOnAxis(ap=ci[:, gc:gc + 1], axis=0))
                nc.tensor.matmul(out=ps_xt[:], lhsT=oh[:], rhs=xc[:],
                                 start=(gc == 0), stop=(gc == 3), skip_group_check=True)
                nc.tensor.matmul(out=ps_vb[:], lhsT=oh[:], rhs=cv[:, gc:gc + 1],
                                 start=(gc == 0), stop=(gc == 3), skip_group_check=True)
            tanhv = wk.tile([128, 1], dt32)
            nc.scalar.activation(tanhv[:], ps_vb[:], AF.Tanh)
            xtl = wk.tile([128, 128], dt32)
            nc.vector.tensor_scalar_mul(xtl[:], ps_xt[:], tanhv[:, :1])

            # ---- GRU step
            ps_t = ps1.tile([128, 128], dt32, tag="p1")
            nc.tensor.transpose(out=ps_t[:], in_=xtl[:], identity=ident[:])
            xtT = wk.tile([128, 128], dt32)
            nc.vector.tensor_copy(xtT[:], ps_t[:])
            ps_gi = ps1.tile([128, 384], dt32, tag="p1")
            nc.tensor.matmul(out=ps_gi[:], lhsT=xtT[:], rhs=wihT[:], start=True, stop=True)
            ps_gh = ps1.tile([128, 384], dt32, tag="p2")
            nc.tensor.matmul(out=ps_gh[:], lhsT=wT[:], rhs=whhT[:], start=True, stop=True)
            gi = wk.tile([128, 384], dt32)
            nc.vector.tensor_add(gi[:], ps_gi[:], bih[:])
            gh = wk.tile([128, 384], dt32)
            nc.vector.tensor_add(gh[:], ps_gh[:], bhh[:])
            rz = wk.tile([128, 256], dt32)
            nc.vector.tensor_add(rz[:], gi[:, 0:256], gh[:, 0:256])
            nc.scalar.activation(rz[:], rz[:], AF.Sigmoid)
            nn_ = wk.tile([128, 128], dt32)
            nc.vector.tensor_mul(nn_[:], rz[:, 0:128], gh[:, 256:384])
            nc.vector.tensor_add(nn_[:], nn_[:], gi[:, 256:384])
            nc.scalar.activation(nn_[:], nn_[:], AF.Tanh)
            # W_new = (1-z)*n + z*W = n + z*(W - n)
            wmn = wk.tile([128, 128], dt32)
            nc.vector.tensor_sub(wmn[:], wsb[:], nn_[:])
            nc.vector.tensor_mul(wmn[:], rz[:, 128:256], wmn[:])
            wnew = wk.tile([128, 128], dt32)
            nc.vector.tensor_add(wnew[:], nn_[:], wmn[:])

            # ---- dinv tables
            with nc.allow_low_precision(reason="dinv fp16 is plenty"):
                dcsb = scp.tile([128, CPAD], dt16)
                nc.sync.dma_start(dcsb[:], degc_d[:])
                nc.scalar.activation(dcsb[:], dcsb[:], AF.Sqrt, bias=1.0)
                nc.vector.reciprocal(dcsb[:], dcsb[:])
                colloc = scp.tile([128, max_nch], dt16)
                nc.sync.dma_start(colloc[:], coll_d[:])
                dgr = scp.tile([128, max_nch], dt16)
                nc.sync.dma_start(dgr[:], degr_d[:])
                nc.scalar.activation(dgr[:], dgr[:], AF.Sqrt, bias=1.0)
                nc.vector.reciprocal(dgr[:], dgr[:])

            # ---- aggregation over col tiles
            outsb = wk.tile([128, CTILES * 4], dt32)
            for T in range(CTILES * agg_reps):
                T = T % CTILES
                ch0 = T * tile_nch
                nch = tile_nch
                pspre = ps.tile([128, 512], dt32, tag="pspre")
                nc.vector.memset(pspre[:], 0.0)
                payt = payp.tile([128, nch, F], dt16, tag="payt")
                nc.sync.dma_start(payt[:], pay_d[:, ch0:ch0 + nch, :])
                mv = mskp.tile([128, nch, W_WIN], dt16, tag="mv")
                nc.vector.tensor_tensor(
                    out=mv[:],
                    in0=colloc[:, ch0:ch0 + nch].to_broadcast([128, nch, W_WIN]),
                    in1=iota3[:], op=OP.is_equal)
                nc.vector.tensor_tensor(
                    out=mv[:], in0=mv[:],
                    in1=dgr[:, ch0:ch0 + nch].to_broadcast([128, nch, W_WIN]),
                    op=OP.mult)
                for k in range(nch):
                    lo = (k // K) * W_WIN
                    nc.tensor.matmul(out=pspre[:, lo:lo + W_WIN],
                                     lhsT=payt[:, k, :], rhs=mv[:, k, :],
                                     start=False, stop=(k == nch - 1),
                                     skip_group_check=True)
                aggpre = wk.tile([128, 512], dt32, tag="aggpre")
                nc.vector.tensor_tensor(out=aggpre[:], in0=pspre[:],
                                        in1=dcsb[:, T * 512:(T + 1) * 512],
                                        op=OP.mult)
                psagg = ps.tile([128, 512], dt32, tag="psagg")
                nc.tensor.matmul(out=psagg[:], lhsT=wnew[:], rhs=aggpre[:],
                                 start=True, stop=True)
                relu = wk.tile([128, 512], dt32, tag="relu")
                nc.scalar.activation(relu[:], psagg[:], AF.Relu, bias=cb[:])
                pshead = ps.tile([128, 4], dt32, tag="ps4")
                nc.tensor.matmul(out=pshead[:], lhsT=ones[:], rhs=lb[:],
                                 start=True, stop=False, skip_group_check=True)
                for m in range(4):
                    nc.tensor.matmul(out=pshead[:, m:m + 1],
                                     lhsT=relu[:, m * 128:(m + 1) * 128],
                                     rhs=lw[:], start=False, stop=(m == 3),
                                     skip_group_check=True)
                nc.vector.tensor_copy(outsb[:, T * 4:(T + 1) * 4], pshead[:])

            nc.sync.dma_start(out_d[:], outsb[:])

    nc.compile()
    return nc


# ---------------------------------------------------------------- runner
def _run(nc, in_maps):
    import jax
    import numpy as _np
    from jax.sharding import Mesh, PartitionSpec, NamedSharding
    from jax.experimental.shard_map import shard_map
    import concourse.mybir as mybir
    from concourse import bass2jax
    from concourse.bass2jax import _bass_exec_p, install_neuronx_cc_hook

    install_neuronx_cc_hook()
    partition_name = nc.partition_id_tensor.name if nc.partition_id_tensor else None
    in_names, out_names, out_avals, zero_outs = [], [], [], []
    for alloc in nc.m.functions[0].allocations:
        if not isinstance(alloc, mybir.MemoryLocationSet):
            continue
        name = alloc.memorylocations[0].name
        if alloc.kind == "ExternalInput":
            if name != partition_name:
                in_names.append(name)
        elif alloc.kind == "ExternalOutput":
            out_names.append(name)
            shape = tuple(alloc.tensor_shape)
            dtype = mybir.dt.np(alloc.dtype)
            out_avals.append(jax.core.ShapedArray(shape, dtype))
            zero_outs.append(_np.zeros(shape, dtype))
    n_params = len(in_names)
    n_outs = len(out_avals)
    all_in = list(in_names) + list(out_names)
    if partition_name is not None:
        all_in.append(partition_name)
    donate = tuple(range(n_params, n_params + n_outs))

    def _body(*args):
        operands = list(args)
        if partition_name is not None:
            operands.append(bass2jax.partition_id_tensor())
        return tuple(_bass_exec_p.bind(
            *operands, out_avals=tuple(out_avals), in_names=tuple(all_in),
            out_names=tuple(out_names), lowering_input_output_aliases=(),
            sim_require_finite=False, sim_require_nnan=False, nc=nc))

    devices = jax.devices()[:NC]
    mesh = Mesh(_np.asarray(devices), ("core",))
    sharded = jax.jit(
        shard_map(_body, mesh=mesh,
                  in_specs=(PartitionSpec("core"),) * (n_params + n_outs),
                  out_specs=(PartitionSpec("core"),) * len(out_names),
                  check_rep=False),
        donate_argnums=donate, keep_unused=True)

    dbg = nc.dbg_addr.name if nc.dbg_addr is not None else None
    if dbg is not None:
        in_maps = [{**m, dbg: _np.zeros((1, 2), _np.uint32)} for m in in_maps]
    per_core = [[_np.asarray(m[nm]) for nm in in_names] for m in in_maps]
    concat_in = [
        _np.ascontiguousarray(_np.concatenate(
            [per_core[c][i] for c in range(NC)], axis=0))
        for i in range(n_params)]
    concat_zeros = [_np.zeros((NC * z.shape[0], *z.shape[1:]), z.dtype)
                    for z in zero_outs]
    out_arrs = sharded(*concat_in, *concat_zeros)
    return [
        {name: _np.asarray(out_arrs[i]).reshape(NC, *out_avals[i].shape)[c]
         for i, name in enumerate(out_names)}
        for c in range(NC)]


_CACHE = {}


def kernel(**inputs) -> np.ndarray:
    per_core, consts = _prep(inputs)
    K = consts.pop("K")
    nc = _build(K)

    in_maps = []
    for c in range(NC):
        m = dict(per_core[c])
        m.update(consts)
        in_maps.append(m)

    res = _run(nc, in_maps)
    out = np.empty((N, 1), np.float32)
    for c in range(NC):
        o = res[c]["out"]                      # [128, 100]
        out[c * SHARD:(c + 1) * SHARD, 0] = o.T.reshape(-1)[:SHARD]
    return out
